# revision 2
# baseline (speedup 1.0000x reference)
"""Trainium2 Bass kernel for nn_AttentionLayer (masked-diagonal attention).

Computes, per (bs, sq) group of n=64 tokens:
  x2 = layernorm(x) (ddof=1, +eps on std); q = x2 Wq^T + bq; k = x2 Wk^T + bk
  per head h: S_h = q_h k_h^T / 8, masked -> softmax rows -> take diagonal,
  sum over heads.

Distribution: data-parallel over the 512 (bs, sq) groups across 8 NeuronCores
(64 groups/core).  Each core processes its groups as 32 "pairs" (2 groups =
128 rows), in 4 superblocks of 8 pairs.

Key observations used:
 - only softmax row-sums Z and the diagonal P_ii are needed, never the full
   normalized attention matrix;
 - no max-subtraction is needed: scores/8 are O(+-4), exp() is f32-safe, and
   masked entries (score-65536) underflow to exactly 0 like the reference's
   exp(-10000-max);
 - rows with mask_i=False produce exactly 0.25 in the reference (uniform
   softmax diag 1/64 x 16 heads), fixed up on the host;
 - LN alpha/bias fold exactly into the projection weights/biases on the host.
"""

import sys

sys.path.insert(0, "/opt/trn_rl_repo")

import numpy as np
import ml_dtypes

import concourse.bass as bass
import concourse.bacc as bacc
import concourse.mybir as mybir
from concourse import tile
from concourse.bass_utils import run_bass_kernel_spmd

F32 = mybir.dt.float32
BF16 = mybir.dt.bfloat16
AF = mybir.ActivationFunctionType
ALU = mybir.AluOpType

DIM = 1024
HEADS = 16
D_K = 64
N_TOK = 64          # tokens per (bs, sq) group
EPS = 1e-6
N_CORES = 8
N_GROUPS = 512      # bs*sq
GROUPS_PER_CORE = N_GROUPS // N_CORES      # 64
PAIRS_PER_CORE = GROUPS_PER_CORE // 2      # 32
SB_PAIRS = 8                               # pairs per superblock
MASK_NEG = -65536.0                        # exactly representable in bf16


def build_graph(n_pairs=PAIRS_PER_CORE, sb_pairs=SB_PAIRS):
    """Build the per-core Bacc graph (SPMD: all cores run the same NEFF)."""
    assert n_pairs % sb_pairs == 0
    n_sb = n_pairs // sb_pairs
    rows_sb = 128 * sb_pairs  # rows per superblock

    nc = bacc.Bacc(None, target_bir_lowering=False)

    x_d = nc.declare_dram_parameter("x", [n_pairs, 128, DIM], F32, isOutput=False)
    wq_d = nc.declare_dram_parameter("wqt", [8, 128, DIM], BF16, isOutput=False)
    wk_d = nc.declare_dram_parameter("wkt", [8, 128, DIM], BF16, isOutput=False)
    bqk_d = nc.declare_dram_parameter("bqk", [2, 8, 128], F32, isOutput=False)
    id_d = nc.declare_dram_parameter("ident", [128, DIM], BF16, isOutput=False)
    bcol_d = nc.declare_dram_parameter("bcol", [n_pairs, 512], BF16, isOutput=False)
    ones_d = nc.declare_dram_parameter("ones", [1, 128], BF16, isOutput=False)
    out_d = nc.declare_dram_parameter("out", [n_pairs, 128, 2], F32, isOutput=True)

    with tile.TileContext(nc) as tc:
        with (
            tc.tile_pool(name="const", bufs=1) as constp,
            tc.tile_pool(name="xin", bufs=3) as xinp,
            tc.tile_pool(name="x2bf", bufs=3) as x2p,
            tc.tile_pool(name="x2t", bufs=1) as x2tp,
            tc.tile_pool(name="qkt", bufs=1) as qktp,
            tc.tile_pool(name="stats", bufs=8) as statp,
            tc.tile_pool(name="psb", bufs=3) as psbp,
            tc.tile_pool(name="zd", bufs=8) as zdp,
            tc.tile_pool(name="res", bufs=4) as resp,
            tc.tile_pool(name="mmps", bufs=4, space=bass.MemorySpace.PSUM) as mmpsp,
            tc.tile_pool(name="scps", bufs=3, space=bass.MemorySpace.PSUM) as scpsp,
        ):
            # ---- constants ----
            wq_sb = constp.tile([128, 8 * DIM], BF16, tag="wq")
            wk_sb = constp.tile([128, 8 * DIM], BF16, tag="wk")
            for kt in range(8):
                nc.sync.dma_start(wq_sb[:, kt * DIM:(kt + 1) * DIM], wq_d[kt])
                nc.sync.dma_start(wk_sb[:, kt * DIM:(kt + 1) * DIM], wk_d[kt])
            bqk_sb = constp.tile([128, 16], F32, tag="bqk")
            for pj in range(2):
                for mt in range(8):
                    nc.gpsimd.dma_start(
                        bqk_sb[:, pj * 8 + mt: pj * 8 + mt + 1], bqk_d[pj, mt][:, None]
                    )
            id_sb = constp.tile([128, DIM], BF16, tag="ident")
            nc.gpsimd.dma_start(id_sb[:], id_d[:])
            bcol_sb = constp.tile([1, n_pairs * 512], BF16, tag="bcol")
            nc.gpsimd.dma_start(bcol_sb[:], bcol_d[:].rearrange("t c -> (t c)")[None, :])
            ones_sb = constp.tile([1, 128], BF16, tag="ones")
            nc.gpsimd.dma_start(ones_sb[:], ones_d[:])

            for sb in range(n_sb):
                x2t = x2tp.tile([128, 8 * rows_sb // 128 * 128], BF16, tag="x2t")
                # layout: x2t[p = kin%128, (kt*sb_pairs + tl)*128 + r]
                for tl in range(sb_pairs):
                    t = sb * sb_pairs + tl
                    xin = xinp.tile([128, DIM], F32, tag="xin")
                    nc.sync.dma_start(xin[:], x_d[t])
                    # -- LN stats (pop var; ddof fixed in sqrt scale) --
                    bno = statp.tile([128, 12], F32, tag="bno")
                    nc.vector.bn_stats(bno[:, 0:6], xin[:, 0:512])
                    nc.vector.bn_stats(bno[:, 6:12], xin[:, 512:1024])
                    mv = statp.tile([128, 2], F32, tag="mv")
                    nc.vector.bn_aggr(mv[:], bno[:])
                    std = statp.tile([128, 1], F32, tag="std")
                    nc.scalar.activation(
                        std[:], mv[:, 1:2], AF.Sqrt, scale=float(DIM) / (DIM - 1)
                    )
                    inv = statp.tile([128, 1], F32, tag="inv")
                    nc.vector.tensor_scalar_add(std[:], std[:], EPS)
                    nc.vector.reciprocal(inv[:], std[:])
                    nmi = statp.tile([128, 1], F32, tag="nmi")
                    # nmi = (mean * -1) * inv
                    nc.vector.scalar_tensor_tensor(
                        nmi[:], mv[:, 0:1], -1.0, inv[:], op0=ALU.mult, op1=ALU.mult
                    )
                    x2 = x2p.tile([128, DIM], BF16, tag="x2")
                    nc.scalar.activation(
                        x2[:], xin[:], AF.Identity, bias=nmi[:], scale=inv[:]
                    )
                    for c in range(8):
                        nc.sync.dma_start(
                            x2t[:, (c * sb_pairs + tl) * 128:(c * sb_pairs + tl + 1) * 128],
                            x2[:, c * 128:(c + 1) * 128],
                            transpose=True,
                        )

                # -- projections: qT/kT = W' @ x2T (+bias on copy-out) --
                qt_sb = qktp.tile([128, 8 * rows_sb], BF16, tag="qt")
                kt_sb = qktp.tile([128, 8 * rows_sb], BF16, tag="kt")
                n_half = rows_sb // 512
                for mt in range(8):
                    for pj, (w_sb, dst) in enumerate(((wq_sb, qt_sb), (wk_sb, kt_sb))):
                        for half in range(n_half):
                            ps = mmpsp.tile([128, 512], F32, tag="mmps")
                            for kt in range(8):
                                nc.tensor.matmul(
                                    ps[:],
                                    w_sb[:, kt * DIM + mt * 128: kt * DIM + (mt + 1) * 128],
                                    x2t[:, kt * rows_sb + half * 512: kt * rows_sb + (half + 1) * 512],
                                    start=(kt == 0),
                                    stop=(kt == 7),
                                )
                            dsl = dst[:, mt * rows_sb + half * 512: mt * rows_sb + (half + 1) * 512]
                            bias_ap = bqk_sb[:, pj * 8 + mt: pj * 8 + mt + 1]
                            if pj == 0:
                                nc.vector.tensor_scalar_add(dsl, ps[:], bias_ap)
                            else:
                                nc.scalar.activation(
                                    dsl, ps[:], AF.Identity, bias=bias_ap, scale=1.0
                                )

                # -- scores + softmax stats per pair --
                for tl in range(sb_pairs):
                    t = sb * sb_pairs + tl
                    psb = psbp.tile([128, DIM], BF16, tag="psb")
                    for half in range(2):
                        ps = scpsp.tile([128, 512], F32, tag="scps")
                        nc.tensor.matmul(
                            ps[:],
                            ones_sb[:],
                            bcol_sb[:, t * 512:(t + 1) * 512],
                            start=True,
                            stop=False,
                            skip_group_check=True,
                        )
                        for mtl in range(4):
                            mt = half * 4 + mtl
                            for hp in range(2):
                                for g in range(2):
                                    r0 = mt * rows_sb + tl * 128 + g * 64
                                    nc.tensor.matmul(
                                        ps[hp * 64:hp * 64 + 64,
                                           mtl * 128 + g * 64: mtl * 128 + g * 64 + 64],
                                        qt_sb[hp * 64:hp * 64 + 64, r0:r0 + 64],
                                        kt_sb[hp * 64:hp * 64 + 64, r0:r0 + 64],
                                        start=False,
                                        stop=True,
                                        skip_group_check=True,
                                    )
                        nc.scalar.activation(
                            psb[:, half * 512:(half + 1) * 512], ps[:], AF.Exp,
                            scale=0.125,
                        )
                    z = zdp.tile([128, 16], F32, tag="z")
                    nc.vector.tensor_reduce(
                        z[:], psb[:].rearrange("p (b j) -> p b j", j=64),
                        axis=mybir.AxisListType.X, op=ALU.add,
                    )
                    pd = psbp.tile([128, DIM], BF16, tag="pd")
                    nc.vector.tensor_mul(pd[:], psb[:], id_sb[:])
                    d = zdp.tile([128, 16], F32, tag="d")
                    nc.vector.tensor_reduce(
                        d[:], pd[:].rearrange("p (b j) -> p b j", j=64),
                        axis=mybir.AxisListType.X, op=ALU.add,
                    )
                    rz = zdp.tile([128, 16], F32, tag="rz")
                    nc.vector.reciprocal(rz[:], z[:])
                    cb = zdp.tile([128, 16], F32, tag="cb")
                    nc.vector.tensor_mul(cb[:], d[:], rz[:])
                    res = resp.tile([128, 2], F32, tag="res")
                    nc.vector.tensor_reduce(
                        res[:], cb[:].rearrange("p (m g) -> p g m", g=2),
                        axis=mybir.AxisListType.X, op=ALU.add,
                    )
                    nc.gpsimd.dma_start(out_d[t], res[:])

    nc.compile()
    return nc


def prepare_host_inputs(x, mask, alpha, bias, Wq, bq, Wk, bk,
                        n_pairs=PAIRS_PER_CORE, n_cores=N_CORES):
    """Fold LN affine params into weights, shard, build per-core in_maps."""
    x = np.asarray(x, np.float32)
    mask = np.asarray(mask, bool)
    alpha = np.asarray(alpha, np.float64)
    bias = np.asarray(bias, np.float64)
    Wq = np.asarray(Wq, np.float64)
    Wk = np.asarray(Wk, np.float64)
    bq = np.asarray(bq, np.float64)
    bk = np.asarray(bk, np.float64)

    Wqp = Wq * alpha[None, :]
    Wkp = Wk * alpha[None, :]
    bqp = (bq + Wq @ bias).astype(np.float32)
    bkp = (bk + Wk @ bias).astype(np.float32)

    wqt = np.ascontiguousarray(
        Wqp.T.reshape(8, 128, DIM).astype(ml_dtypes.bfloat16))
    wkt = np.ascontiguousarray(
        Wkp.T.reshape(8, 128, DIM).astype(ml_dtypes.bfloat16))
    bqk = np.stack([bqp.reshape(8, 128), bkp.reshape(8, 128)])

    ident = np.zeros((128, DIM), ml_dtypes.bfloat16)
    j = np.arange(DIM) % 64
    p = np.arange(128) % 64
    ident[p[:, None] == j[None, :]] = 1.0

    ones = np.ones((1, 128), ml_dtypes.bfloat16)

    n_groups = x.size // (N_TOK * DIM)
    xg = x.reshape(n_groups, N_TOK, DIM)
    mg = mask.reshape(n_groups, N_TOK)
    gpc = 2 * n_pairs
    in_maps = []
    for c in range(n_cores):
        xs = np.ascontiguousarray(
            xg[c * gpc:(c + 1) * gpc].reshape(n_pairs, 128, DIM))
        ms = mg[c * gpc:(c + 1) * gpc].reshape(n_pairs, 128)
        bcol = np.where(np.tile(ms, (1, 4)), 0.0, MASK_NEG).astype(ml_dtypes.bfloat16)
        in_maps.append({
            "x": xs, "wqt": wqt, "wkt": wkt, "bqk": bqk,
            "ident": ident, "bcol": np.ascontiguousarray(bcol), "ones": ones,
        })
    return in_maps


def postprocess(results, mask, n_pairs=PAIRS_PER_CORE, n_cores=N_CORES):
    """Gather per-core results, sum head-parity halves, apply mask fixup."""
    mask = np.asarray(mask, bool)
    out = np.empty((N_GROUPS, N_TOK), np.float32)
    gpc = 2 * n_pairs
    for c in range(n_cores):
        res = results[c]["out"]                       # [n_pairs, 128, 2]
        summed = res[:, 0:64, :] + res[:, 64:128, :]  # [n_pairs, 64(i), 2(g)]
        out[c * gpc:(c + 1) * gpc] = summed.transpose(0, 2, 1).reshape(gpc, N_TOK)
    out = out.reshape(mask.shape)
    out[~mask] = 0.25
    return out


_NC_CACHE = {}


def _get_graph():
    if "nc" not in _NC_CACHE:
        _NC_CACHE["nc"] = build_graph()
    return _NC_CACHE["nc"]


def kernel(x, mask, alpha, bias, Wq, bq, Wk, bk, _trace=False, _trace_kwargs=None):
    nc = _get_graph()
    in_maps = prepare_host_inputs(x, mask, alpha, bias, Wq, bq, Wk, bk)
    kw = {}
    if _trace:
        kw = dict(trace=True, **(_trace_kwargs or {}))
    r = run_bass_kernel_spmd(nc, in_maps, core_ids=list(range(N_CORES)), **kw)
    out = postprocess(r.results, mask)
    if _trace:
        kernel.last_exec_time_ns = r.exec_time_ns
        kernel.last_results = r
    return out


# revision 6
# speedup vs baseline: 1.4648x; 1.4648x over previous
"""Trainium2 Bass kernel for nn_AttentionLayer (masked-diagonal attention).

Computes, per (bs, sq) group of n=64 tokens:
  x2 = layernorm(x) (ddof=1, +eps on std); q = x2 Wq^T + bq; k = x2 Wk^T + bk
  per head h: S_h = q_h k_h^T / 8, masked -> softmax rows -> take diagonal,
  sum over heads.

Distribution: data-parallel over the 512 (bs, sq) groups across 8 NeuronCores
(64 groups/core).  Each core processes its groups as 32 "pairs" (2 groups =
128 rows), in 4 superblocks of 8 pairs.

Key observations used:
 - only softmax row-sums Z and the diagonal P_ii are needed, never the full
   normalized attention matrix;
 - no max-subtraction is needed: scores/8 are O(+-4), exp() is f32-safe, and
   masked entries (score-65536) underflow to exactly 0 like the reference's
   exp(-10000-max);
 - rows with mask_i=False produce exactly 0.25 in the reference (uniform
   softmax diag 1/64 x 16 heads), fixed up on the host;
 - LN alpha/bias fold exactly into the projection weights/biases on the host.
"""

import sys

sys.path.insert(0, "/opt/trn_rl_repo")

import numpy as np
import ml_dtypes

import concourse.bass as bass
import concourse.bacc as bacc
import concourse.mybir as mybir
from concourse import tile
from concourse.bass_utils import run_bass_kernel_spmd

F32 = mybir.dt.float32
BF16 = mybir.dt.bfloat16
AF = mybir.ActivationFunctionType
ALU = mybir.AluOpType

DIM = 1024
HEADS = 16
D_K = 64
N_TOK = 64          # tokens per (bs, sq) group
EPS = 1e-6
N_CORES = 8
N_GROUPS = 512      # bs*sq
GROUPS_PER_CORE = N_GROUPS // N_CORES      # 64
PAIRS_PER_CORE = GROUPS_PER_CORE // 2      # 32
SB_PAIRS = 8                               # pairs per superblock
MASK_NEG = -65536.0                        # exactly representable in bf16


def build_graph(n_pairs=PAIRS_PER_CORE, sb_pairs=SB_PAIRS):
    """Build the per-core Bacc graph (SPMD: all cores run the same NEFF)."""
    assert n_pairs % sb_pairs == 0
    n_sb = n_pairs // sb_pairs
    rows_sb = 128 * sb_pairs  # rows per superblock

    nc = bacc.Bacc(None, target_bir_lowering=False)

    x_d = nc.declare_dram_parameter("x", [n_pairs, 128, DIM], F32, isOutput=False)
    wq_d = nc.declare_dram_parameter("wqt", [8, 128, DIM], BF16, isOutput=False)
    wk_d = nc.declare_dram_parameter("wkt", [8, 128, DIM], BF16, isOutput=False)
    bqk_d = nc.declare_dram_parameter("bqk", [2, 8, 128], F32, isOutput=False)
    id_d = nc.declare_dram_parameter("ident", [128, DIM], BF16, isOutput=False)
    bcol_d = nc.declare_dram_parameter("bcol", [n_pairs, 512], BF16, isOutput=False)
    ones_d = nc.declare_dram_parameter("ones", [1, 128], BF16, isOutput=False)
    out_d = nc.declare_dram_parameter("out", [n_pairs, 128, 2], F32, isOutput=True)

    with tile.TileContext(nc) as tc:
        with (
            tc.tile_pool(name="const", bufs=1) as constp,
            tc.tile_pool(name="xin", bufs=3) as xinp,
            tc.tile_pool(name="x2bf", bufs=3) as x2p,
            tc.tile_pool(name="x2t", bufs=2) as x2tp,
            tc.tile_pool(name="qkt", bufs=2) as qktp,
            tc.tile_pool(name="stats", bufs=8) as statp,
            tc.tile_pool(name="psb", bufs=3) as psbp,
            tc.tile_pool(name="zd", bufs=8) as zdp,
            tc.tile_pool(name="res", bufs=4) as resp,
            tc.tile_pool(name="mmps", bufs=2, space=bass.MemorySpace.PSUM) as mmpsp,
            tc.tile_pool(name="scps", bufs=4, space=bass.MemorySpace.PSUM) as scpsp,
        ):
            # ---- constants ----
            wq_sb = constp.tile([128, 8 * DIM], BF16, tag="wq")
            wk_sb = constp.tile([128, 8 * DIM], BF16, tag="wk")
            for kt in range(8):
                nc.sync.dma_start(wq_sb[:, kt * DIM:(kt + 1) * DIM], wq_d[kt])
                nc.sync.dma_start(wk_sb[:, kt * DIM:(kt + 1) * DIM], wk_d[kt])
            bqk_sb = constp.tile([128, 16], F32, tag="bqk")
            for pj in range(2):
                for mt in range(8):
                    nc.gpsimd.dma_start(
                        bqk_sb[:, pj * 8 + mt: pj * 8 + mt + 1], bqk_d[pj, mt][:, None]
                    )
            id_sb = constp.tile([128, DIM], BF16, tag="ident")
            nc.gpsimd.dma_start(id_sb[:], id_d[:])
            bcol_sb = constp.tile([1, n_pairs * 512], BF16, tag="bcol")
            nc.gpsimd.dma_start(bcol_sb[:], bcol_d[:].rearrange("t c -> (t c)")[None, :])
            ones_sb = constp.tile([1, 128], BF16, tag="ones")
            nc.gpsimd.dma_start(ones_sb[:], ones_d[:])

            for sb in range(n_sb):
                x2t = x2tp.tile([128, 8 * rows_sb // 128 * 128], BF16, tag="x2t")
                # layout: x2t[p = kin%128, (kt*sb_pairs + tl)*128 + r]
                for tl in range(sb_pairs):
                    t = sb * sb_pairs + tl
                    xin = xinp.tile([128, DIM], F32, tag="xin")
                    nc.gpsimd.dma_start(xin[:], x_d[t])
                    # -- LN stats (pop var; ddof fixed in sqrt scale) --
                    bno = statp.tile([128, 12], F32, tag="bno")
                    nc.vector.bn_stats(bno[:, 0:6], xin[:, 0:512])
                    nc.vector.bn_stats(bno[:, 6:12], xin[:, 512:1024])
                    mv = statp.tile([128, 2], F32, tag="mv")
                    nc.vector.bn_aggr(mv[:], bno[:])
                    std = statp.tile([128, 1], F32, tag="std")
                    nc.scalar.activation(
                        std[:], mv[:, 1:2], AF.Sqrt, scale=float(DIM) / (DIM - 1)
                    )
                    inv = statp.tile([128, 1], F32, tag="inv")
                    nc.vector.tensor_scalar_add(std[:], std[:], EPS)
                    nc.vector.reciprocal(inv[:], std[:])
                    nmi = statp.tile([128, 1], F32, tag="nmi")
                    # nmi = (mean * -1) * inv
                    nc.vector.scalar_tensor_tensor(
                        nmi[:], mv[:, 0:1], -1.0, inv[:], op0=ALU.mult, op1=ALU.mult
                    )
                    x2 = x2p.tile([128, DIM], BF16, tag="x2")
                    nc.scalar.activation(
                        x2[:], xin[:], AF.Identity, bias=nmi[:], scale=inv[:]
                    )
                    # one batched xbar transpose: out[p, c, r] = x2[r, c*128+p]
                    x2t_4d = x2t[:].rearrange(
                        "p (c t r) -> p c t r", c=8, t=sb_pairs, r=128
                    )
                    nc.sync.dma_start(x2t_4d[:, :, tl, :], x2[:], transpose=True)

                # -- projections: qT/kT = W' @ x2T (+bias on copy-out) --
                qt_sb = qktp.tile([128, 8 * rows_sb], BF16, tag="qt")
                kt_sb = qktp.tile([128, 8 * rows_sb], BF16, tag="kt")
                n_half = rows_sb // 512
                for mt in range(8):
                    for pj, (w_sb, dst) in enumerate(((wq_sb, qt_sb), (wk_sb, kt_sb))):
                        ps = mmpsp.tile([128, rows_sb], F32, tag="mmps")
                        for kt in range(8):
                            for half in range(n_half):
                                nc.tensor.matmul(
                                    ps[:, half * 512:(half + 1) * 512],
                                    w_sb[:, kt * DIM + mt * 128: kt * DIM + (mt + 1) * 128],
                                    x2t[:, kt * rows_sb + half * 512: kt * rows_sb + (half + 1) * 512],
                                    start=(kt == 0),
                                    stop=(kt == 7),
                                )
                        dsl = dst[:, mt * rows_sb:(mt + 1) * rows_sb]
                        bias_ap = bqk_sb[:, pj * 8 + mt: pj * 8 + mt + 1]
                        if pj == 0:
                            nc.vector.tensor_scalar_add(dsl, ps[:], bias_ap)
                        else:
                            nc.scalar.activation(
                                dsl, ps[:], AF.Identity, bias=bias_ap, scale=1.0
                            )

                # -- scores + softmax stats per pair --
                for tl in range(sb_pairs):
                    t = sb * sb_pairs + tl
                    psb = psbp.tile([128, DIM], BF16, tag="psb")
                    for half in range(2):
                        ps = scpsp.tile([128, 512], F32, tag="scps")
                        nc.tensor.matmul(
                            ps[:],
                            ones_sb[:],
                            bcol_sb[:, t * 512:(t + 1) * 512],
                            start=True,
                            stop=False,
                            skip_group_check=True,
                        )
                        for mtl in range(4):
                            mt = half * 4 + mtl
                            for hp in range(2):
                                for g in range(2):
                                    r0 = mt * rows_sb + tl * 128 + g * 64
                                    nc.tensor.matmul(
                                        ps[hp * 64:hp * 64 + 64,
                                           mtl * 128 + g * 64: mtl * 128 + g * 64 + 64],
                                        qt_sb[hp * 64:hp * 64 + 64, r0:r0 + 64],
                                        kt_sb[hp * 64:hp * 64 + 64, r0:r0 + 64],
                                        start=False,
                                        stop=True,
                                        skip_group_check=True,
                                    )
                        nc.scalar.activation(
                            psb[:, half * 512:(half + 1) * 512], ps[:], AF.Exp,
                            scale=0.125,
                        )
                    z = zdp.tile([128, 16], F32, tag="z")
                    nc.vector.tensor_reduce(
                        z[:], psb[:].rearrange("p (b j) -> p b j", j=64),
                        axis=mybir.AxisListType.X, op=ALU.add,
                    )
                    pd = psbp.tile([128, DIM], BF16, tag="pd")
                    nc.vector.tensor_mul(pd[:], psb[:], id_sb[:])
                    d = zdp.tile([128, 16], F32, tag="d")
                    nc.vector.tensor_reduce(
                        d[:], pd[:].rearrange("p (b j) -> p b j", j=64),
                        axis=mybir.AxisListType.X, op=ALU.add,
                    )
                    rz = zdp.tile([128, 16], F32, tag="rz")
                    nc.vector.reciprocal(rz[:], z[:])
                    cb = zdp.tile([128, 16], F32, tag="cb")
                    nc.vector.tensor_mul(cb[:], d[:], rz[:])
                    res = resp.tile([128, 2], F32, tag="res")
                    nc.vector.tensor_reduce(
                        res[:], cb[:].rearrange("p (m g) -> p g m", g=2),
                        axis=mybir.AxisListType.X, op=ALU.add,
                    )
                    nc.gpsimd.dma_start(out_d[t], res[:])

    nc.compile()
    return nc


def prepare_host_inputs(x, mask, alpha, bias, Wq, bq, Wk, bk,
                        n_pairs=PAIRS_PER_CORE, n_cores=N_CORES):
    """Fold LN affine params into weights, shard, build per-core in_maps."""
    x = np.asarray(x, np.float32)
    mask = np.asarray(mask, bool)
    alpha = np.asarray(alpha, np.float64)
    bias = np.asarray(bias, np.float64)
    Wq = np.asarray(Wq, np.float64)
    Wk = np.asarray(Wk, np.float64)
    bq = np.asarray(bq, np.float64)
    bk = np.asarray(bk, np.float64)

    Wqp = Wq * alpha[None, :]
    Wkp = Wk * alpha[None, :]
    bqp = (bq + Wq @ bias).astype(np.float32)
    bkp = (bk + Wk @ bias).astype(np.float32)

    wqt = np.ascontiguousarray(
        Wqp.T.reshape(8, 128, DIM).astype(ml_dtypes.bfloat16))
    wkt = np.ascontiguousarray(
        Wkp.T.reshape(8, 128, DIM).astype(ml_dtypes.bfloat16))
    bqk = np.stack([bqp.reshape(8, 128), bkp.reshape(8, 128)])

    ident = np.zeros((128, DIM), ml_dtypes.bfloat16)
    j = np.arange(DIM) % 64
    p = np.arange(128) % 64
    ident[p[:, None] == j[None, :]] = 1.0

    ones = np.ones((1, 128), ml_dtypes.bfloat16)

    n_groups = x.size // (N_TOK * DIM)
    xg = x.reshape(n_groups, N_TOK, DIM)
    mg = mask.reshape(n_groups, N_TOK)
    gpc = 2 * n_pairs
    in_maps = []
    for c in range(n_cores):
        xs = np.ascontiguousarray(
            xg[c * gpc:(c + 1) * gpc].reshape(n_pairs, 128, DIM))
        ms = mg[c * gpc:(c + 1) * gpc].reshape(n_pairs, 128)
        bcol = np.where(np.tile(ms, (1, 4)), 0.0, MASK_NEG).astype(ml_dtypes.bfloat16)
        in_maps.append({
            "x": xs, "wqt": wqt, "wkt": wkt, "bqk": bqk,
            "ident": ident, "bcol": np.ascontiguousarray(bcol), "ones": ones,
        })
    return in_maps


def postprocess(results, mask, n_pairs=PAIRS_PER_CORE, n_cores=N_CORES):
    """Gather per-core results, sum head-parity halves, apply mask fixup."""
    mask = np.asarray(mask, bool)
    out = np.empty((N_GROUPS, N_TOK), np.float32)
    gpc = 2 * n_pairs
    for c in range(n_cores):
        res = results[c]["out"]                       # [n_pairs, 128, 2]
        summed = res[:, 0:64, :] + res[:, 64:128, :]  # [n_pairs, 64(i), 2(g)]
        out[c * gpc:(c + 1) * gpc] = summed.transpose(0, 2, 1).reshape(gpc, N_TOK)
    out = out.reshape(mask.shape)
    out[~mask] = 0.25
    return out


_NC_CACHE = {}


def _get_graph():
    if "nc" not in _NC_CACHE:
        _NC_CACHE["nc"] = build_graph()
    return _NC_CACHE["nc"]


def kernel(x, mask, alpha, bias, Wq, bq, Wk, bk, _trace=False, _trace_kwargs=None):
    nc = _get_graph()
    in_maps = prepare_host_inputs(x, mask, alpha, bias, Wq, bq, Wk, bk)
    kw = {}
    if _trace:
        kw = dict(trace=True, **(_trace_kwargs or {}))
    r = run_bass_kernel_spmd(nc, in_maps, core_ids=list(range(N_CORES)), **kw)
    out = postprocess(r.results, mask)
    if _trace:
        kernel.last_exec_time_ns = r.exec_time_ns
        kernel.last_results = r
    return out
